# revision 1
# baseline (speedup 1.0000x reference)
"""Trainium2 Bass kernel: per-image Gaussian blur (sigma=3.5, 29-tap, scipy
'reflect' boundary) over H, W and channel axes of [64, 512, 512, 3] images.

Strategy: the blur is linear and separable, so per image
    Y = A_H^T @ X @ B,   X = image as [H=512, W*C=1536]
where A_H is the 512x512 banded (+-14) H-blur matrix with the symmetric
boundary folded in, and B = kron(A_W, M) is the 1536x1536 banded (+-44)
combined W+channel blur matrix over the flattened (w, c) axis.

Both passes run on the TensorEngine with the *image chunk* as the stationary
operand, so each pass transposes orientation for free:
    pass 1: out1[wc, h]  = sum_k X[k-chunk, wc-chunk]^T @ A_H[k-chunk, band]
    pass 2: out2[h, wc]  = sum_k out1[k-chunk, h-chunk]^T @ B[k-chunk, band]
Band structure keeps matmul free dims ~142-512 wide. PSUM accumulation uses
per-element has_written semantics (overlapping band writes).

Default mode "fp16h" (see _build_module): the host pre-casts the input to
fp16 and all TensorE operands are fp16 (f32 PSUM accumulate), the output is
staged/DMA'd as fp16 and converted back to f32 on the host. Rationale, from
measurement: (a) f32r matmuls with moving free-dim < 256 run at 4 cycles/row
on the PE — every matmul here is 44-216 wide, so 16-bit operands are 4x
faster (and walrus rejects mixed f32r x 16-bit operands); (b) the kernel is
bound by the ~300 GB/s/core aggregate DMA bandwidth shared by reads+writes,
so halving both input (12MB/core) and output (12MB/core) traffic halves the
floor. Norm rel err 4.95e-04 (fp16 has a 10-bit mantissa and values are
O(1)), vs the 2e-2 gate.

Sharding: pure data parallel, 64 images -> 8 per NeuronCore.
"""

import numpy as np

SIGMA = 3.5
R = 14  # truncate 4.0 * 3.5 + 0.5 -> 14
B_TOTAL, H, W, C = 64, 512, 512, 3
WC = W * C
N_CORES = 8
B_LOCAL = B_TOTAL // N_CORES
P = 128
BAND_WC = 3 * R + C - 1  # 44

# sim_safe=True makes the first matmul touching each PSUM bank cover the whole
# bank so CoreSim's all-or-none pending-zero assert holds. Hardware supports
# the cheaper overlapping-band writes (per-element has_written), default False.
SIM_SAFE = False

_MODULE_CACHE = {}
_MATS_CACHE = {}


# ---------------------------------------------------------------- matrices

def _gauss_weights():
    x = np.arange(-R, R + 1, dtype=np.float64)
    w = np.exp(-0.5 * (x / SIGMA) ** 2)
    return w / w.sum()


def _axis_matrix(L):
    w = _gauss_weights()
    idx = np.pad(np.arange(L), R, mode="symmetric")
    A = np.zeros((L, L), dtype=np.float64)
    for o in range(L):
        for t in range(2 * R + 1):
            A[idx[o + t], o] += w[t]
    return A


def _pass1_pieces(sim_safe):
    pieces = []
    for k in range(4):
        s = max(0, 128 * k - R)
        e = min(H, 128 * k + 128 + R)
        if k == 0 and sim_safe:
            s, e = 0, H
        pieces.append((k, s, e, k == 0, k == 3))
    return pieces


def _pass2_pieces(sim_safe):
    bank_pieces = {0: [], 1: [], 2: []}
    for k in range(WC // 128):
        s = max(0, 128 * k - BAND_WC)
        e = min(WC, 128 * k + 128 + BAND_WC)
        b0, b1 = s // 512, (e - 1) // 512
        for b in range(b0, b1 + 1):
            ps, pe = max(s, 512 * b), min(e, 512 * (b + 1))
            if sim_safe and not bank_pieces[b]:
                ps, pe = 512 * b, 512 * (b + 1)
            bank_pieces[b].append([k, ps, pe, False, False])
    for b in range(3):
        bank_pieces[b][0][3] = True   # start
        bank_pieces[b][-1][4] = True  # stop
    return bank_pieces


def _build_mats(sim_safe):
    if sim_safe in _MATS_CACHE:
        return _MATS_CACHE[sim_safe]
    A_H = _axis_matrix(H).astype(np.float32)
    Bm = np.kron(_axis_matrix(W), _axis_matrix(C)).astype(np.float32)

    # pack A_H chunks: [128, 4*512], chunk k at cols [512k, 512k+512)
    ah_packed = np.zeros((P, 4 * H), dtype=np.float32)
    for k in range(4):
        ah_packed[:, 512 * k:512 * (k + 1)] = A_H[128 * k:128 * k + 128, :]

    # pack B chunk windows
    bp = _pass2_pieces(sim_safe)
    windows = {}
    for b in range(3):
        for (k, s, e, _, _) in bp[b]:
            w0, w1 = windows.get(k, (s, e))
            windows[k] = (min(w0, s), max(w1, e))
    offs, off = {}, 0
    for k in range(WC // 128):
        w0, w1 = windows[k]
        offs[k] = off
        off += w1 - w0
    bw_packed = np.zeros((P, off), dtype=np.float32)
    for k in range(WC // 128):
        w0, w1 = windows[k]
        bw_packed[:, offs[k]:offs[k] + (w1 - w0)] = Bm[128 * k:128 * k + 128, w0:w1]

    _MATS_CACHE[sim_safe] = (ah_packed, bw_packed, windows, offs, bp)
    return _MATS_CACHE[sim_safe]


# ---------------------------------------------------------------- bass module

# inq/outq pick the DMA-issuing engine (whose sequencer is held for the
# whole transfer): 0 = Activation, 1 = SP (sync), 2 = Pool (gpsimd SWDGE),
# 3 = DVE (vector)
# inq=2: loads issue from the idle Pool (SWDGE) queue so they never queue
# behind the 4 per-image out-issues on SP (in-order sequencer) — measured
# 91.7 vs 95.4us in-batch. Same-dtype SWDGE transfer; the casting SWDGE
# load path was correctness-verified in the fp16/bf16 modes.
TUNE = {"xin": 3, "mid": 3, "ostage": 3, "ps1": 4, "ps2": 4, "ldwopt": 0,
        "outq": 1, "inq": 2, "pipe": 0,
        # p2order: emit pass-2 units bank-major so the earliest units only
        # depend on the first ~5 pass-1 copies (PE flows pass1->pass2 with
        # no head-of-line stall on the copy drain)
        "p2order": 0,
        # pair1: two pass-1 wc-chunks share one 2-bank PSUM tile + one
        # double-width copy (halves pass-1 copy instruction count)
        "pair1": 0,
        # osplit: stage the output in per-m-group tiles and fire each
        # group's out-DMA as soon as it is staged (finer DMA interleave,
        # shorter copy-tail before each out; ~2.5us better than one
        # whole-image out-DMA, measured in-batch)
        "osplit": 4,
        # cpool: rotate PSUM->SBUF copies over three engines (DVE, ACT,
        # Pool) instead of two — the Pool engine is idle in fp16h mode
        "cpool": 0,
        # isplit: issue the per-image in-DMA as 2 or 4 piecewise transfers
        # (by h-chunk) so a long in-transfer can't head-of-line block a
        # ready out-transfer on the non-preemptible DMA engines
        "isplit": 0}

# Runtime switch consulted by the walrus-arg patch: when on, compiles run
# with --enable-ldw-opt=true (separate LDWEIGHTS the PE can hoist; only
# sound for bf16 operands -- broken for f32/f32r).
_LDWOPT_STATE = {"on": False}


def _install_ldwopt_patch():
    import concourse.bass_utils as bu
    if getattr(bu, "_ldwopt_patched", False):
        return
    orig = bu.run_command

    def patched(argv, **kw):
        if _LDWOPT_STATE["on"]:
            argv = ["--enable-ldw-opt=true" if a == "--enable-ldw-opt=false"
                    else a for a in argv]
        return orig(argv, **kw)

    bu.run_command = patched
    bu._ldwopt_patched = True


def _build_module(sim_safe, bench_reps=0, variant="full", mmdt="f32r",
                  tune=None):
    """mmdt picks the TensorE operand dtype:
    - "f32": true fp32 — 4 passes through the PE array (slowest, ~1.6e-7)
    - "f32r": FP22-truncated fp32 — single pass (~2e-4 error). NOTE: the PE
      runs f32r matmuls with moving free-dim < 256 at 4 cycles/row (SBUF
      read bandwidth); all matmuls here are 44-216 wide, so this mode is
      4x slower than bf16 on the PE.
    - "bf16": bf16 operands, f32 PSUM accumulate (~3.4e-3 error); inputs are
      cast during the gpsimd (SWDGE) load, matrices pre-cast on host
    - "fp16": like "bf16" but float16 operands AND float16 output staging/
      DMA (host converts back to f32). Same PE speed (1 cycle/row), 8x
      smaller rounding error than bf16 (10-bit vs 7-bit mantissa; values
      are O(1) so the reduced exponent range is harmless), and the fp16
      output DMA halves the output HBM traffic.
    - "fp16h": "fp16" with the input pre-cast to fp16 on the HOST, so the
      device reads 12MB instead of 24MB per core and the in-DMA is a plain
      HWDGE transfer (no SWDGE cast). Numerically identical to "fp16"
      (the input is rounded to fp16 either way). DMA traffic per core
      drops to 12MB in + 12MB out = 24MB (~79us at the ~304GB/s measured
      aggregate DMA rate).
    NOTE: mixing f32r with 16-bit operands is rejected by the walrus
    birverifier (checkMatmultInputs: if either operand is f32/f32r, both
    transfer types must match), so the image must be cast on load.
    """
    tune = dict(TUNE, **(tune or {}))
    key = (sim_safe, bench_reps, variant, mmdt, tuple(sorted(tune.items())))
    if key in _MODULE_CACHE:
        return _MODULE_CACHE[key]

    import concourse.mybir as mybir
    import concourse.tile as tile
    from concourse import bacc

    ah_packed, bw_packed, windows, offs, bank_pieces = _build_mats(sim_safe)
    p1 = _pass1_pieces(sim_safe)
    f32 = mybir.dt.float32
    f32r = mybir.dt.float32r
    bf16dt = mybir.dt.bfloat16
    fp16dt = mybir.dt.float16
    # float32r tiles: DMA'd bytes are raw fp32 (PE truncates to FP22);
    # compute-produced tiles (x1 copies) are rounded by the producing engine.
    # per-mode dtypes: (matrix sbuf, image sbuf, output sbuf+dram)
    mat_sb, img_sb, out_dt = {
        "f32": (f32, f32, f32),
        "f32r": (f32r, f32r, f32),
        "bf16": (bf16dt, bf16dt, f32),
        "fp16": (fp16dt, fp16dt, fp16dt),
        "fp16h": (fp16dt, fp16dt, fp16dt),
    }[mmdt]
    bf16 = mmdt in ("bf16", "fp16")  # SWDGE cast-on-load of the image
    host_in16 = mmdt == "fp16h"  # input arrives in DRAM already fp16
    mat_host_cast = mat_sb in (bf16dt, fp16dt)  # host pre-casts matrices

    def mm(out_ap, lhs_ap, rhs_ap, start, stop):
        nc.tensor.matmul(out_ap, lhs_ap, rhs_ap, start=start, stop=stop)

    nc = bacc.Bacc("TRN2", debug=False, enable_asserts=False, num_devices=N_CORES)
    x_dram_dt = fp16dt if host_in16 else f32
    x_d = nc.dram_tensor("x", (B_LOCAL, H, WC), x_dram_dt,
                         kind="ExternalInput").ap()
    mat_dt = mat_sb if mat_host_cast else f32
    ah_d = nc.dram_tensor("ah", ah_packed.shape, mat_dt, kind="ExternalInput").ap()
    bw_d = nc.dram_tensor("bw", bw_packed.shape, mat_dt, kind="ExternalInput").ap()
    y_d = nc.dram_tensor("y", (B_LOCAL, H, WC), out_dt, kind="ExternalOutput").ap()

    with tile.TileContext(nc) as tc:
        with tc.tile_pool(name="const", bufs=1) as cpool, \
             tc.tile_pool(name="xin", bufs=tune["xin"]) as xpool, \
             tc.tile_pool(name="mid", bufs=tune["mid"]) as mpool, \
             tc.tile_pool(name="ostage", bufs=tune["ostage"]) as opool, \
             tc.tile_pool(name="ps1",
                          bufs=(tune["ps1"] // 2 if tune["pair1"]
                                else tune["ps1"]),
                          space="PSUM") as ps1pool, \
             tc.tile_pool(name="ps2", bufs=tune["ps2"], space="PSUM") as ps2pool:

            if tune["ldwopt"]:
                # marker op: make the BIR differ so no compile cache can
                # serve a NEFF built with the other walrus flag setting
                mk = cpool.tile([P, 8], f32, tag="ldwopt_marker", name="ldwm")
                nc.vector.memset(mk[:], 0.0)
            ah_t = cpool.tile([P, ah_packed.shape[1]], mat_sb, tag="ah", name="ah_t")
            bw_t = cpool.tile([P, bw_packed.shape[1]], mat_sb, tag="bw", name="bw_t")
            # consts on the Activation queue: overlaps image 0's in-DMA
            # (which runs on the sync queue) during the one-shot warmup
            if mat_host_cast:
                nc.scalar.dma_start(ah_t[:], ah_d[:])
                nc.scalar.dma_start(bw_t[:], bw_d[:])
            else:
                nc.scalar.dma_start(ah_t[:], ah_d[:].bitcast(mat_sb))
                nc.scalar.dma_start(bw_t[:], bw_d[:].bitcast(mat_sb))

            engs = {0: nc.scalar, 1: nc.sync, 2: nc.gpsimd, 3: nc.vector}
            out_eng = engs[tune["outq"]]
            in_eng = engs[tune["inq"]]

            def copy_to(dst, src, idx):
                if tune["cpool"]:
                    r = idx % 3
                    if r == 0:
                        nc.vector.tensor_copy(dst, src)
                    elif r == 1:
                        nc.scalar.copy(dst, src)
                    else:
                        nc.gpsimd.tensor_copy(dst, src)
                elif idx % 2 == 1:
                    nc.scalar.copy(dst, src)
                else:
                    nc.vector.tensor_copy(dst, src)

            def emit_load(img):
                xt = xpool.tile([P, 4 * WC], img_sb, tag="x", name=f"x_{img}")
                if host_in16:
                    isplit = tune["isplit"]
                    if isplit:
                        ksz = 4 // isplit  # h-chunks per piece
                        for g in range(isplit):
                            in_eng.dma_start(
                                xt[:, WC * ksz * g:WC * ksz * (g + 1)]
                                .rearrange("p (k n) -> p k n", n=WC),
                                x_d[img][128 * ksz * g:128 * ksz * (g + 1)]
                                .rearrange("(k p) n -> p k n", p=P))
                    else:
                        x_src = x_d[img].rearrange("(k p) n -> p k n", p=P)
                        in_eng.dma_start(
                            xt[:].rearrange("p (k n) -> p k n", n=WC), x_src)
                elif bf16:
                    x_src = x_d[img].rearrange("(k p) n -> p k n", p=P)
                    nc.gpsimd.dma_start(
                        xt[:].rearrange("p (k n) -> p k n", n=WC), x_src)
                else:
                    x_src = x_d[img].rearrange("(k p) n -> p k n", p=P).bitcast(img_sb)
                    nc.sync.dma_start(
                        xt[:].rearrange("p (k n) -> p k n", n=WC), x_src)
                return xt

            def emit_pass1(img, xt):
                """H-blur. With pair1, two wc-chunks share one 2-bank PSUM
                tile and one (larger) PSUM->SBUF copy. Returns lhs(k, m):
                an AP for x1 chunk k, h-columns [128m, 128m+128)."""
                if tune["pair1"]:
                    x1 = []
                    for j in range(WC // 256):  # pair (2j, 2j+1)
                        ps = ps1pool.tile([P, 2 * H], f32, tag="ps1",
                                          name=f"ps1_{img}_{j}")
                        for half in range(2):
                            m = 2 * j + half
                            for (k, s, e, start, stop) in p1:
                                mm(
                                    ps[:, H * half + s:H * half + e],
                                    xt[:, WC * k + 128 * m:WC * k + 128 * (m + 1)],
                                    ah_t[:, 512 * k + s:512 * k + e],
                                    start, stop,
                                )
                        t1 = mpool.tile([P, 2 * H], img_sb, tag=f"m{j}",
                                        name=f"x1_{img}_{j}")
                        copy_to(t1[:], ps[:], j)
                        x1.append(t1)

                    def lhs(k, m):
                        return x1[k // 2][:, H * (k % 2) + 128 * m:
                                          H * (k % 2) + 128 * (m + 1)]
                    return lhs

                x1 = []
                for m in range(WC // 128):
                    ps = ps1pool.tile([P, H], f32, tag="ps1", name=f"ps1_{img}_{m}")
                    for (k, s, e, start, stop) in p1:
                        mm(
                            ps[:, s:e],
                            xt[:, WC * k + 128 * m:WC * k + 128 * (m + 1)],
                            ah_t[:, 512 * k + s:512 * k + e],
                            start, stop,
                        )
                    t1 = mpool.tile([P, H], img_sb, tag=f"m{m}", name=f"x1_{img}_{m}")
                    copy_to(t1[:], ps[:], m)
                    x1.append(t1)

                def lhs(k, m):
                    return x1[k][:, 128 * m:128 * (m + 1)]
                return lhs

            def emit_pass2(img, lhs):
                osplit = tune["osplit"]  # 0=off, 2 or 4 = way-split out-DMA
                if osplit:
                    gsz = 4 // osplit  # m-chunks per out-DMA group
                    ots = [opool.tile([P, gsz * WC], out_dt, tag=f"o{g}",
                                      name=f"o_{img}_{g}")
                           for g in range(osplit)]
                else:
                    ot = opool.tile([P, 4 * WC], out_dt, tag="o", name=f"o_{img}")
                units = ([(m, b) for b in range(3) for m in range(4)]
                         if tune["p2order"] else
                         [(m, b) for m in range(4) for b in range(3)])
                for (m, b) in units:
                    ps = ps2pool.tile([P, 512], f32, tag="ps2",
                                      name=f"ps2_{img}_{m}_{b}")
                    for (k, s, e, start, stop) in bank_pieces[b]:
                        w0 = windows[k][0]
                        mm(
                            ps[:, s - 512 * b:e - 512 * b],
                            lhs(k, m),
                            bw_t[:, offs[k] + s - w0:offs[k] + e - w0],
                            start, stop,
                        )
                    if osplit:
                        dst = ots[m // gsz][:, WC * (m % gsz) + 512 * b:
                                            WC * (m % gsz) + 512 * (b + 1)]
                    else:
                        dst = ot[:, WC * m + 512 * b:WC * m + 512 * (b + 1)]
                    copy_to(dst, ps[:], m + b)
                    if osplit and b == 2 and (m + 1) % gsz == 0:
                        # group staged: fire its out-DMA now
                        g = m // gsz
                        out_eng.dma_start(
                            y_d[img][128 * gsz * g:128 * gsz * (g + 1)]
                            .rearrange("(k p) n -> p k n", p=P),
                            ots[g][:].rearrange("p (k n) -> p k n", n=WC))
                if not osplit:
                    y_dst = y_d[img].rearrange("(k p) n -> p k n", p=P)
                    out_eng.dma_start(
                        y_dst, ot[:].rearrange("p (k n) -> p k n", n=WC))

            def emit_image(img):
                if variant == "inonly_hw":
                    # timing bisection: plain f32 HWDGE load, no cast
                    xt32 = xpool.tile([P, 4 * WC], f32, tag="x32",
                                      name=f"x32_{img}")
                    nc.sync.dma_start(
                        xt32[:].rearrange("p (k n) -> p k n", n=WC),
                        x_d[img].rearrange("(k p) n -> p k n", p=P))
                    return
                xt = emit_load(img)

                if variant == "inonly":
                    return
                y_dst = y_d[img].rearrange("(k p) n -> p k n", p=P)

                if variant == "dmaonly":
                    # timing bisection: stream in + out, no compute
                    src = xt[:] if out_dt == img_sb else xt[:].bitcast(f32)
                    out_eng.dma_start(
                        y_dst, src.rearrange("p (k n) -> p k n", n=WC))
                    return

                if variant == "full":
                    emit_pass2(img, emit_pass1(img, xt))
                    return

                # mmonly2: every matmul emitted twice (PE-speed probe; the
                # doubled accumulation garbles values, timing-only variant)
                mmreps = 2 if variant == "mmonly2" else 1

                # pass 1: out1[wc-chunk m] = [128, 512(h)]
                x1 = []
                for m in range(WC // 128):
                    ps = ps1pool.tile([P, H], f32, tag="ps1", name=f"ps1_{img}_{m}")
                    for r in range(mmreps):
                        for (k, s, e, start, stop) in p1:
                            mm(
                                ps[:, s:e],
                                xt[:, WC * k + 128 * m:WC * k + 128 * (m + 1)],
                                ah_t[:, 512 * k + s:512 * k + e],
                                start and r == 0, stop and r == mmreps - 1,
                            )
                    if variant in ("nocopy", "mmonly", "mmonly2"):
                        continue
                    t1 = mpool.tile([P, H], img_sb, tag=f"m{m}", name=f"x1_{img}_{m}")
                    if m % 2 == 1:
                        nc.scalar.copy(t1[:], ps[:])
                    else:
                        nc.vector.tensor_copy(t1[:], ps[:])
                    x1.append(t1)

                # pass 2: out2[h-chunk m] at cols [1536m, 1536m+1536) of the
                # staged output tile; ONE 3MB DMA out on the scalar HWDGE ring
                # (separate FIFO from the input ring -> latencies overlap).
                ot = opool.tile([P, 4 * WC], out_dt, tag="o", name=f"o_{img}")
                raw_lhs = variant in ("nocopy", "mmonly", "mmonly2")
                for m in range(4):
                    for b in range(3):
                        ps = ps2pool.tile([P, 512], f32, tag="ps2",
                                          name=f"ps2_{img}_{m}_{b}")
                        for r in range(mmreps):
                            for (k, s, e, start, stop) in bank_pieces[b]:
                                w0 = windows[k][0]
                                lhs = (xt[:, WC * (k % 4) + 128 * m:
                                          WC * (k % 4) + 128 * (m + 1)]
                                       if raw_lhs else
                                       x1[k][:, 128 * m:128 * (m + 1)])
                                mm(
                                    ps[:, s - 512 * b:e - 512 * b],
                                    lhs,
                                    bw_t[:, offs[k] + s - w0:offs[k] + e - w0],
                                    start and r == 0, stop and r == mmreps - 1,
                                )
                        if raw_lhs:
                            continue
                        dst = ot[:, WC * m + 512 * b:WC * m + 512 * (b + 1)]
                        if (m + b) % 2 == 1:
                            nc.scalar.copy(dst, ps[:])
                        else:
                            nc.vector.tensor_copy(dst, ps[:])
                if variant in ("mmonly", "mmonly2"):
                    return  # no out-DMA: isolates PE + in-DMA
                if variant == "nocopy":
                    src = xt[:] if out_dt == img_sb else xt[:].bitcast(f32)
                else:
                    src = ot[:]
                out_eng.dma_start(
                    y_dst, src.rearrange("p (k n) -> p k n", n=WC))

            def emit_all():
                if tune["pipe"] and variant == "full":
                    # software pipeline: emit pass2(i) AFTER pass1(i+1) so
                    # the in-order PE queue never head-of-line blocks on
                    # image i's pass-1 PSUM->SBUF copies
                    pending = None
                    for img in range(B_LOCAL):
                        xt = emit_load(img)
                        x1 = emit_pass1(img, xt)
                        if pending is not None:
                            emit_pass2(*pending)
                        pending = (img, x1)
                    emit_pass2(*pending)
                else:
                    for img in range(B_LOCAL):
                        emit_image(img)

            if bench_reps:
                ET = mybir.EngineType
                with tc.For_i(0, bench_reps, 1,
                              hint_engines=(ET.PE, ET.DVE, ET.Activation,
                                            ET.SP, ET.Pool)):
                    emit_all()
            else:
                emit_all()

    nc.compile()
    _MODULE_CACHE[key] = nc
    return nc


# ---------------------------------------------------------------- entry points

def _run(images, trace=False, sim_safe=None, mmdt="fp16h", **trace_kwargs):
    from concourse import bass_utils

    if sim_safe is None:
        sim_safe = SIM_SAFE
    nc = _build_module(sim_safe, mmdt=mmdt)
    ah_packed, bw_packed, _, _, _ = _build_mats(sim_safe)
    if mmdt == "bf16":
        import ml_dtypes
        ah_packed = ah_packed.astype(ml_dtypes.bfloat16)
        bw_packed = bw_packed.astype(ml_dtypes.bfloat16)
    elif mmdt in ("fp16", "fp16h"):
        ah_packed = ah_packed.astype(np.float16)
        bw_packed = bw_packed.astype(np.float16)

    imgs = np.ascontiguousarray(np.asarray(images, dtype=np.float32)
                                .reshape(B_TOTAL, H, WC))
    if mmdt == "fp16h":
        imgs = imgs.astype(np.float16)
    in_maps = [
        {
            "x": imgs[c * B_LOCAL:(c + 1) * B_LOCAL],
            "ah": ah_packed,
            "bw": bw_packed,
        }
        for c in range(N_CORES)
    ]
    res = bass_utils.run_bass_kernel_spmd(
        nc, in_maps, core_ids=list(range(N_CORES)), trace=trace, **trace_kwargs
    )
    out = np.concatenate(
        [np.asarray(res.results[c]["y"], dtype=np.float32)
         .reshape(B_LOCAL, H, W, C) for c in range(N_CORES)],
        axis=0,
    )
    return out, res


def kernel(images, original_shapes=None, **_ignored):
    # original_shapes is always the full frame (crop = identity) per the
    # reference problem; it is unused.
    out, _ = _run(images, trace=False)
    return out

